# revision 17
# baseline (speedup 1.0000x reference)
"""GGNN message passing (gnn_message_passing) Trainium2 Bass kernel.

Problem (hardcoded, self-contained):
  node_state [32, 1024, 64] f32, adj_mat [32, 1024, 1024] i32 (values 0..3),
  matrix_in/matrix_out [4, 64, 64] f32, bias [128] f32.
  out[b,i,:64]  = sum_j matrix_in [adj[b,i,j]] @ h[b,j] + bias[:64]
  out[b,i,64:]  = sum_j matrix_out[adj[b,j,i]] @ h[b,j] + bias[64:]

Strategy: data-parallel over batch, 4 batches per core on 8 cores.

Math per batch (classes c=1..3 via fp8 {0,1} masks, class 0 via ones-trick):
  m_in  = sum_c A_c  @ P'in_c  + ones*(hsum @ M0in.T  + bias_in )
  m_out = sum_c A_c.T @ P'out_c + ones*(hsum @ M0out.T + bias_out)
  where P'dir_c = h @ (Mdir_c - Mdir_0).T computed on the PE.
P' is carried as an exact fp8 hi/lo pair (~2^-8 relative), masks are
exact in fp8, so the masked matmuls run in fp8 DoubleRow perf mode
(two 128-row j-tiles contracted per instruction).
Key layout trick: the whole [1024,1024] adjacency transpose is ONE
DMA-xbar instruction into a [q, ti, tj, p] blocked layout (vt2).
Outputs are produced transposed ([dir, d, i]) and fixed up on the host.
"""
import sys

sys.path.insert(0, "/opt/trn_rl_repo")

import numpy as np
import ml_dtypes

from concourse import bacc, bass, mybir, tile
from concourse.bass_utils import run_bass_kernel_spmd

bf16 = ml_dtypes.bfloat16
dt = mybir.dt
Alu = mybir.AluOpType
DR = mybir.MatmulPerfMode.DoubleRow

NCORES = 8
BATCH = 32
BPC = BATCH // NCORES  # batches per core
N = 1024
D = 64
NT = N // 128  # 8 row-tiles
NU = NT // 2  # 4 j-tile pairs (DoubleRow contracts two tiles per MM)


def build_program(reps=1, split_waits=True):
    del split_waits
    nc = bacc.Bacc("TRN2", target_bir_lowering=False, debug=False)

    adj_d = nc.dram_tensor("adj", [BPC, N, N], dt.int32, kind="ExternalInput")
    h_d = nc.dram_tensor("h", [BPC, N, D], dt.float32, kind="ExternalInput")
    # [hi/lo, (split,e)=128, (dir,c,d)=384]
    mbig_d = nc.dram_tensor("mbig", [2, 128, 384], dt.bfloat16, kind="ExternalInput")
    # [hi/lo, (split,e)=128, (dir,d)=128]
    m0big_d = nc.dram_tensor("m0big", [2, 128, 128], dt.bfloat16, kind="ExternalInput")
    biasr_d = nc.dram_tensor("biasr", [1, 128], dt.float32, kind="ExternalInput")
    out_d = nc.dram_tensor("out", [BPC, 2, D, N], dt.float32, kind="ExternalOutput")

    with tile.TileContext(nc) as tc:
        with (
            tc.tile_pool(name="consts", bufs=1) as cpool,
            tc.tile_pool(name="adj32", bufs=2) as adj_pool,
            tc.tile_pool(name="v", bufs=2) as v_pool,
            tc.tile_pool(name="vt", bufs=2) as vt_pool,
            tc.tile_pool(name="hin", bufs=2) as h_pool,
            tc.tile_pool(name="hbi", bufs=2) as hbi_pool,
            tc.tile_pool(name="ht", bufs=2) as ht_pool,
            tc.tile_pool(name="pbuf", bufs=2) as p_pool,
            tc.tile_pool(name="mask", bufs=10) as mask_pool,
            tc.tile_pool(name="mt", bufs=2) as mt_pool,
            tc.tile_pool(name="small", bufs=2) as s_pool,
            tc.tile_pool(name="psP", bufs=2, space="PSUM") as psP_pool,
            tc.tile_pool(name="psM", bufs=2, space="PSUM") as psM_pool,
            tc.tile_pool(name="psS", bufs=1, space="PSUM") as psS_pool,
            tc.tile_pool(name="psT", bufs=1, space="PSUM") as psT_pool,
        ):
            # ---- constants ----
            mbig_t = cpool.tile([128, 2, 384], dt.bfloat16)
            nc.sync.dma_start(mbig_t[:, 0, :], mbig_d[0])
            nc.sync.dma_start(mbig_t[:, 1, :], mbig_d[1])
            m0big_t = cpool.tile([128, 2, 128], dt.bfloat16)
            nc.sync.dma_start(m0big_t[:, 0, :], m0big_d[0])
            nc.sync.dma_start(m0big_t[:, 1, :], m0big_d[1])
            biasr_t = cpool.tile([1, 128], dt.float32)
            nc.sync.dma_start(biasr_t[:], biasr_d[:])
            onescol = cpool.tile([128, 1], dt.bfloat16)
            nc.vector.memset(onescol[:], 1.0)
            ones512 = cpool.tile([1, 512], dt.bfloat16)
            nc.vector.memset(ones512[:], 1.0)

            # ---- software-pipelined emission helpers ----
            # Engines execute their streams in order, so batch b+1's
            # load->cast->transpose chain must be EMITTED inside batch b's
            # compute, or the PE stalls ~15-20us at every batch boundary.
            def emit_loads(b):
                adj_r = adj_d[b].rearrange("(t p) j -> p t j", p=128)
                chunks = []
                for ck in range(2):
                    adj32 = adj_pool.tile([128, NT // 2, N], dt.int32)
                    nc.sync.dma_start(adj32[:], adj_r[:, 4 * ck : 4 * ck + 4, :])
                    chunks.append(adj32)
                h_t = h_pool.tile([128, NT, D], dt.float32)
                nc.scalar.dma_start(
                    h_t[:], h_d[b].rearrange("(t p) e -> p t e", p=128)
                )
                return {"chunks": chunks, "h": h_t}

            def emit_xform(L):
                # h hi/lo + ht FIRST (the next batch's first PE work, psP,
                # needs ht), then cast int32->bf16 (DVE) + transpose strips.
                h_t = L["h"]
                hbi_t = hbi_pool.tile([128, NT, 128], dt.bfloat16)
                nc.vector.tensor_copy(hbi_t[:, :, 0:D], h_t[:])
                nc.vector.tensor_tensor(
                    hbi_t[:, :, D:128], h_t[:], hbi_t[:, :, 0:D], Alu.subtract
                )
                ht_t = ht_pool.tile([128, NT, 128], dt.bfloat16)
                nc.sync.dma_start_transpose(ht_t[:, :, :], hbi_t[:, :, :])
                v_t = v_pool.tile([128, NT, N], dt.bfloat16)
                vt_t = vt_pool.tile([128, NT, N], dt.bfloat16)
                for ck in range(2):
                    nc.vector.tensor_copy(
                        v_t[:, 4 * ck : 4 * ck + 4, :], L["chunks"][ck][:]
                    )
                    # vt_t[q, tj, ti*128+p] = adj[ti*128+p, tj*128+q]; per-J
                    # slices vt_t[:, J, :] stay contiguous for fast DVE reads.
                    for ti in range(4 * ck, 4 * ck + 4):
                        nc.sync.dma_start_transpose(
                            vt_t[:, :, ti * 128 : (ti + 1) * 128],
                            v_t[:, ti, :],
                        )
                return {"v": v_t, "vt": vt_t, "hbi": hbi_t, "ht": ht_t}

            total = BPC * reps
            L = emit_loads(0)
            X = emit_xform(L)
            Xnext = None
            for b_ in range(total):
                b = b_ % BPC
                v_t, vt_t = X["v"], X["vt"]
                hbi_t, ht_t = X["hbi"], X["ht"]
                Lnext = emit_loads((b_ + 1) % BPC) if b_ + 1 < total else None

                # ---- P' = h @ B.T for all (dir, k), exact via bf16 hi/lo ----
                # pbuf[:, J, m, 0:64]=bf16 hi, [64:128]=lo;  m = dir*3 + k
                pbuf_t = p_pool.tile([128, NT, 6, 128], dt.bfloat16)
                for J in range(NT):
                    psP = psP_pool.tile([128, 384], dt.float32)
                    nc.tensor.matmul(
                        psP[:], ht_t[:, J, :], mbig_t[:, 0, :], start=True, stop=False
                    )
                    nc.tensor.matmul(
                        psP[:], ht_t[:, J, :], mbig_t[:, 1, :], start=False, stop=True
                    )
                    psP_v = psP[:].rearrange("p (m d) -> p m d", m=6)
                    nc.scalar.copy(pbuf_t[:, J, :, 0:D], psP_v)
                    nc.vector.tensor_tensor(
                        pbuf_t[:, J, :, D:128], psP_v, pbuf_t[:, J, :, 0:D], Alu.subtract
                    )

                # ---- hsum = sum_j h  (as exact hi/lo pair per (split,e)) ----
                psS = psS_pool.tile([128, 1], dt.float32)
                for J in range(NT):
                    nc.tensor.matmul(
                        psS[:], hbi_t[:, J, :], onescol[:],
                        start=(J == 0), stop=(J == NT - 1),
                    )
                hs32 = s_pool.tile([128, 1], dt.float32)
                nc.scalar.copy(hs32[:], psS[:])
                hsh = s_pool.tile([128, 1], dt.bfloat16)
                nc.vector.tensor_copy(hsh[:], hs32[:])
                hsl = s_pool.tile([128, 1], dt.bfloat16)
                nc.vector.tensor_tensor(hsl[:], hs32[:], hsh[:], Alu.subtract)

                # ---- t0 = hsum @ M0.T + bias, as bf16 hi/lo pair ----
                psT = psT_pool.tile([1, 128], dt.float32)
                nc.tensor.matmul(psT[:], hsh[:], m0big_t[:, 0, :], start=True, stop=False)
                nc.tensor.matmul(psT[:], hsl[:], m0big_t[:, 0, :], start=False, stop=False)
                nc.tensor.matmul(psT[:], hsh[:], m0big_t[:, 1, :], start=False, stop=False)
                nc.tensor.matmul(psT[:], hsl[:], m0big_t[:, 1, :], start=False, stop=True)
                t0f = s_pool.tile([1, 128], dt.float32)
                nc.scalar.copy(t0f[:], psT[:])
                nc.vector.tensor_tensor(t0f[:], t0f[:], biasr_t[:], Alu.add)
                t0b = s_pool.tile([1, 2, 128], dt.bfloat16)
                t0f_v = t0f[:].rearrange("p (a d) -> p a d", a=2)
                nc.vector.tensor_copy(t0b[:, :, 0:D], t0f_v)
                nc.vector.tensor_tensor(t0b[:, :, D:128], t0f_v, t0b[:, :, 0:D], Alu.subtract)

                # ---- stage-1: m.T accumulation over basis streams ----
                # Streams per (dir, J): {raw values, (v==2), (v==3)} — the raw
                # adjacency (already bf16 in v_t / vt2) is stream k=0, so only
                # two is_equal passes per (dir, J). Stationaries B1..B3 are
                # basis-changed on the host to match.
                # dir 0 (in): streams from vt2 ; dir 1 (out): streams from v_t
                psM = [
                    psM_pool.tile([128, N], dt.float32, tag="psm", name=f"psm{b_}_{d_}")
                    for d_ in range(2)
                ]
                # dir 1 first: it streams v_t (ready right after the casts),
                # giving the vt strips of this batch extra time to land.
                for dir_ in (1, 0):
                    src_t = vt_t if dir_ == 0 else v_t
                    for J in range(NT):
                        streams = [(dir_ * 3, src_t[:, J, :])]  # raw values
                        for k, c in ((1, 2.0), (2, 3.0)):
                            mask_t = mask_pool.tile([128, N], dt.bfloat16)
                            nc.vector.tensor_scalar(
                                mask_t[:], src_t[:, J, :], c, None, Alu.is_equal
                            )
                            streams.append((dir_ * 3 + k, mask_t[:]))
                        for m, stream in streams:
                            for half in range(2):
                                nc.tensor.matmul(
                                    psM[dir_][:, half * 512 : (half + 1) * 512],
                                    pbuf_t[:, J, m, :],
                                    stream[:, half * 512 : (half + 1) * 512],
                                    start=(J == 0 and m % 3 == 0),
                                    stop=False,
                                    skip_group_check=True,
                                )
                    for half in range(2):
                        nc.tensor.matmul(
                            psM[dir_][:, half * 512 : (half + 1) * 512],
                            t0b[:, dir_, :],
                            ones512[:],
                            start=False,
                            stop=True,
                            skip_group_check=True,
                        )
                    if dir_ == 1 and Lnext is not None:
                        # inject next batch's cast+transpose chain here so it
                        # overlaps this batch's second-direction compute
                        Xnext = emit_xform(Lnext)

                # ---- evacuate m.T = hi + lo rows, DMA out ----
                mt_t = mt_pool.tile([D, 2, N], dt.float32)
                for dir_ in range(2):
                    nc.scalar.copy(mt_t[:, dir_, :], psM[dir_][0:D, :])
                    nc.vector.tensor_tensor(
                        mt_t[:, dir_, :], mt_t[:, dir_, :], psM[dir_][D:128, :], Alu.add
                    )
                    nc.scalar.dma_start(out_d[b, dir_], mt_t[:, dir_, :])
                L = Lnext
                if Lnext is not None:
                    X = Xnext

    nc.compile()
    return nc


def host_consts(matrix_in, matrix_out, bias):
    def split(x):
        hi = x.astype(bf16)
        lo = (x - hi.astype(np.float32)).astype(bf16)
        return hi, lo

    # Mbig [ (s,e)=128, (dir,k,d)=384 ]: rows duplicated across split halves.
    # Basis change so streams are {raw v, (v==2), (v==3)} (+ ones trick):
    #   P_c = B0 + c*B1 + 1{c=2}*B2 + 1{c=3}*B3, B0 = M0.
    mb = np.zeros((128, 384), np.float32)
    for dir_, M in ((0, matrix_in), (1, matrix_out)):
        basis = [
            M[1] - M[0],                      # B1 (scaled by raw value)
            M[2] - 2.0 * M[1] + M[0],         # B2 (mask v==2)
            M[3] - 3.0 * M[1] + 2.0 * M[0],   # B3 (mask v==3)
        ]
        for k, Mp in enumerate(basis):
            col = dir_ * 192 + k * 64
            mb[0:64, col : col + 64] = Mp.T  # [e, d]
            mb[64:128, col : col + 64] = Mp.T
    mb_hi, mb_lo = split(mb)
    mbig = np.stack([mb_hi, mb_lo])

    m0 = np.zeros((128, 128), np.float32)
    for dir_, M in ((0, matrix_in), (1, matrix_out)):
        m0[0:64, dir_ * 64 : dir_ * 64 + 64] = M[0].T
        m0[64:128, dir_ * 64 : dir_ * 64 + 64] = M[0].T
    m0_hi, m0_lo = split(m0)
    m0big = np.stack([m0_hi, m0_lo])

    biasr = np.ascontiguousarray(bias.reshape(1, 128).astype(np.float32))
    return mbig, m0big, biasr


class Runner:
    """Cached jitted SPMD executor for one built program (bass2jax path)."""

    def __init__(self, reps=1):
        import jax
        from jax.sharding import Mesh, PartitionSpec
        from jax.experimental.shard_map import shard_map
        from concourse import bass2jax

        self.jax = jax
        bass2jax.install_neuronx_cc_hook()
        nc = build_program(reps)
        self.nc = nc

        partition_name = (
            nc.partition_id_tensor.name if nc.partition_id_tensor else None
        )
        in_names, out_names, out_avals, zero_outs = [], [], [], []
        for alloc in nc.m.functions[0].allocations:
            if not isinstance(alloc, mybir.MemoryLocationSet):
                continue
            name = alloc.memorylocations[0].name
            if alloc.kind == "ExternalInput":
                if name != partition_name:
                    in_names.append(name)
            elif alloc.kind == "ExternalOutput":
                shape = tuple(alloc.tensor_shape)
                np_dt = mybir.dt.np(alloc.dtype)
                out_names.append(name)
                out_avals.append(jax.core.ShapedArray(shape, np_dt))
                zero_outs.append(np.zeros(shape, np_dt))
        self.in_names, self.out_names = in_names, out_names
        self.out_avals, self.zero_outs = out_avals, zero_outs
        n_params, n_outs = len(in_names), len(out_names)
        donate = tuple(range(n_params, n_params + n_outs))

        bind_names = in_names + out_names
        if partition_name is not None:
            bind_names = bind_names + [partition_name]

        def _body(*args):
            operands = list(args)
            if partition_name is not None:
                operands.append(bass2jax.partition_id_tensor())
            outs = bass2jax._bass_exec_p.bind(
                *operands,
                out_avals=tuple(out_avals),
                in_names=tuple(bind_names),
                out_names=tuple(out_names),
                lowering_input_output_aliases=(),
                sim_require_finite=True,
                sim_require_nnan=True,
                nc=nc,
            )
            return tuple(outs)

        devices = jax.devices()[:NCORES]
        mesh = Mesh(np.asarray(devices), ("core",))
        in_specs = (PartitionSpec("core"),) * (n_params + n_outs)
        out_specs = (PartitionSpec("core"),) * n_outs
        self.fn = jax.jit(
            shard_map(
                _body, mesh=mesh, in_specs=in_specs, out_specs=out_specs,
                check_rep=False,
            ),
            donate_argnums=donate,
            keep_unused=True,
        )

    def concat_inputs(self, in_maps):
        return [
            np.concatenate([np.asarray(m[n]) for m in in_maps], axis=0)
            for n in self.in_names
        ]

    def zeros(self):
        return [
            np.zeros((NCORES * z.shape[0], *z.shape[1:]), z.dtype)
            for z in self.zero_outs
        ]

    def __call__(self, concat_in, zeros=None):
        out = self.fn(*concat_in, *(zeros if zeros is not None else self.zeros()))
        return out


_CACHE = {}


def _get_runner(reps=1):
    if reps not in _CACHE:
        _CACHE[reps] = Runner(reps)
    return _CACHE[reps]


def _prep_in_maps(node_state, adj_mat, matrix_in, matrix_out, bias):
    mbig, m0big, biasr = host_consts(matrix_in, matrix_out, bias)
    in_maps = []
    for k in range(NCORES):
        sl = slice(k * BPC, (k + 1) * BPC)
        in_maps.append(
            {
                "adj": np.ascontiguousarray(adj_mat[sl]),
                "h": np.ascontiguousarray(node_state[sl]),
                "mbig": mbig,
                "m0big": m0big,
                "biasr": biasr,
            }
        )
    return in_maps


def _assemble(out_arrs, out_names, out_avals):
    o_all = np.asarray(out_arrs[out_names.index("out")])
    o_all = o_all.reshape(NCORES, *out_avals[out_names.index("out")].shape)
    outs = [
        np.transpose(o_all[k], (0, 3, 1, 2)).reshape(BPC, N, 2 * D)
        for k in range(NCORES)
    ]
    return np.concatenate(outs, 0).astype(np.float32)


def kernel(node_state, adj_mat, matrix_in, matrix_out, bias):
    node_state = np.asarray(node_state, np.float32)
    adj_mat = np.ascontiguousarray(np.asarray(adj_mat, np.int32))
    matrix_in = np.asarray(matrix_in, np.float32)
    matrix_out = np.asarray(matrix_out, np.float32)
    bias = np.asarray(bias, np.float32)

    runner = _get_runner(1)
    in_maps = _prep_in_maps(node_state, adj_mat, matrix_in, matrix_out, bias)
    out_arrs = runner(runner.concat_inputs(in_maps))
    return _assemble(out_arrs, runner.out_names, runner.out_avals)
